# revision 30
# baseline (speedup 1.0000x reference)
"""MultiHeadAttention Trainium2 kernel.

Problem (hardcoded): S=2048, B=2, D=1024, H=16, HD=64, fp32 I/O.
  q = query @ w_q.T + b_q   (same for k, v), heads split from D
  scores[i,j,b,h] = (q_i . k_j)/8, masked where mask[j]==0, softmax over j
  out[i,b,:] = concat_h( sum_j p_ij v_j )

Sharding: 8 cores = 2 batches x 4 head-groups (4 heads / 256 dims each).
Host-side prep: cast to bf16, transpose to [D, seq] layout, and compact the
key/value sequence to the unmasked positions only (masked j contribute
exactly 0 after softmax), padded to a multiple of 128.

Per-core program (Tile framework). The kernel is jointly throughput-limited
by the PE (matmul streams ~84us) and by the 9.4M-element exp. The exp is
split across TWO engines: ACT computes exact exp (~1.09 ns/elem) and the
DVE computes a Schraudolph bit-trick exp (x*c+d -> int16, bitcast bf16,
~1.19 ns/elem, ~1.8% rel err). Only ~45% of tiles go through the DVE path
so the added output error stays ~1.2% (checked vs the 2e-2 gate).

  - PE warm-up: ~48 dummy matmuls during the initial DMA window flip the
    HAM clock gate to 2.4 GHz before the real projections start.
  - DMA order: wk, xk(j 0:512), wq, xq(i 0:1024) first so the first score
    block (ib0, hp0, jt0..3) starts ~14us in; everything else streams
    behind it.
  - Q,K projections k-outer, output qT/kT[o, seq] bf16, o on partitions.
  - V projection flipped (x^T stationary) -> V[j, o] with j on partitions;
    V_ext adds a per-head mask column so the softmax denominator falls out
    of the PV matmul; assembled with one strided DVE copy per j-tile.
  - Scores transposed: S^T[j, i] = kT.T @ qT, two heads row-tiled in the
    128-row PE array (base-partition 0/64) -> pairs overlap ~1.4x.
  - P^T tiles [128, 1024] bf16: ACT exp or DVE Schraudolph per a static
    policy table.
  - PV: out^T[vd, i] += V_ext[j,:].T @ P^T[j, i-chunk], fp32 PSUM over j.
  - Output leaves unnormalized [65, 512] with denominator rows; drains
    alternate DVE/ACT; host folds the softmax division into unsharding.
  - Software-pipelined emission: projections, V waves and PV groups are
    "fillers" placed inside later phases' exp windows.
"""

import math
import sys

sys.path.insert(0, "/opt/trn_rl_repo")

import numpy as np
import ml_dtypes

import concourse.tile as tile
from concourse import bacc, mybir
from concourse.bass_utils import run_bass_kernel_spmd

S, B, D, H, HD = 2048, 2, 1024, 16, 64
N_CORES = 8
GROUPS = 4          # head groups (cores per batch)
GH = H // GROUPS    # heads per core = 4
GD = GH * HD        # dims per core = 256
KT = D // 128       # contraction k-tiles = 8
IBLK = 1024         # i block (exp granularity / P^T tile width)
NIB = S // IBLK     # i blocks = 2
VW1 = HD + 1        # per-head vext width (64 v cols + denominator col)

BF16 = mybir.dt.bfloat16
F32 = mybir.dt.float32
I16 = mybir.dt.int16
EXP = mybir.ActivationFunctionType.Exp
ALU = mybir.AluOpType

LN2 = math.log(2.0)
SCHR_C = 128.0 / LN2             # bf16-bitcast exp: i16 = x*scale*C + D
SCHR_D = (127.0 - 0.0436775) * 128.0

_CACHE = {}


def _chunks(total, step):
    out = []
    o = 0
    while o < total:
        n = min(step, total - o)
        out.append((o, n))
        o += n
    return out


def _pairs(seq):
    return [tuple(seq[i:i + 2]) for i in range(0, len(seq), 2)]


def _exp_policy(njt):
    """engine per (phase_idx, jt, half): True = DVE schraudolph.

    ACT's FIFO queue must stay pure exp (a dependent copy in it blocks
    later exps), so the DVE share is kept small enough that the DVE's
    other work (proj copies, vext, drains) still fits: ~6 tiles/phase."""
    pol = {}
    for p in range(4):
        for jt in range(njt):
            a = b = False
            if p == 0:
                if jt >= 5:
                    b = True                     # 4 DVE tiles
            else:
                if jt % 3 == 1:
                    a = True
                elif jt % 3 == 2:
                    b = True                     # 6 DVE tiles/phase
            pol[(p, jt, 0)] = a
            pol[(p, jt, 1)] = b
    return pol


def _build(J, J_real, use_bias):
    """Build + compile the per-core Bass program (identical on all cores)."""
    NJT = J // 128
    nc = bacc.Bacc("TRN2", target_bir_lowering=False, debug=False,
                   enable_asserts=False)

    # x tensors host-tiled to [128, (block, k, s)] so every DMA block is a
    # fully contiguous 2D transfer (see _x_prep / _x_blocks)
    xq_d = nc.dram_tensor("xq", (128, KT * S), BF16, kind="ExternalInput")
    xk_d = nc.dram_tensor("xk", (128, KT * J), BF16, kind="ExternalInput")
    xv_d = nc.dram_tensor("xv", (128, KT * J), BF16, kind="ExternalInput")
    # weights host-prearranged to the sbuf layout [128, KT*GD]
    wq_d = nc.dram_tensor("wq", (128, KT * GD), BF16, kind="ExternalInput")
    wk_d = nc.dram_tensor("wk", (128, KT * GD), BF16, kind="ExternalInput")
    wv_d = nc.dram_tensor("wv", (128, KT * GD), BF16, kind="ExternalInput")
    mpad_d = nc.dram_tensor("mpad", (128, NJT), BF16, kind="ExternalInput")
    if use_bias:
        bq_d = nc.dram_tensor("bq", (GD, 1), F32, kind="ExternalInput")
        bk_d = nc.dram_tensor("bk", (GD, 1), F32, kind="ExternalInput")
        bv_d = nc.dram_tensor("bv", (1, GD), BF16, kind="ExternalInput")
    out_d = nc.dram_tensor("out", (GH * VW1, S), F32, kind="ExternalOutput")

    # SBUF budget for the P^T pool, in per-partition bytes.
    fixed_pp = (KT * S * 2                 # xq tiles
                + 2 * KT * J * 2           # xk, xv tiles
                + 3 * KT * GD * 2          # weights
                + 2 * S * 2 + 2 * J * 2    # qT/kT pool
                + NJT * (GH * VW1 + 8) * 2   # vext
                + 4 * 512 * 4              # out staging
                + 8 * 1024)                # consts, mpad, warm, slack
    budget_pp = 188 * 1024 - fixed_pp
    pt_bufs = max(NJT + 2, min(4 * NJT + 4, budget_pp // (IBLK * 2)))
    pipelined = pt_bufs >= 4 * NJT

    scale = 1.0 / math.sqrt(HD)  # 0.125, folded into the exp
    policy = _exp_policy(NJT)
    drain_ctr = [0]

    with tile.TileContext(nc) as tc:
        with (
            tc.tile_pool(name="xq", bufs=1) as xq_p,
            tc.tile_pool(name="xk", bufs=1) as xk_p,
            tc.tile_pool(name="xv", bufs=1) as xv_p,
            tc.tile_pool(name="w", bufs=3) as w_p,
            tc.tile_pool(name="qk", bufs=2) as qk_p,
            tc.tile_pool(name="vext", bufs=NJT) as vext_p,
            tc.tile_pool(name="pt", bufs=pt_bufs) as pt_p,
            tc.tile_pool(name="small", bufs=12) as small_p,
            tc.tile_pool(name="ost", bufs=4) as ost_p,
            tc.tile_pool(name="sps", bufs=3, space="PSUM") as sps_p,
            tc.tile_pool(name="pps", bufs=2, space="PSUM") as pps_p,
        ):
            # ---- PE warm-up: flip HAM to 2.4 GHz during the DMA window ----
            warm_w = small_p.tile([128, 128], BF16, tag="warmw")
            nc.vector.memset(warm_w[:], 0.0)
            warm_ps = pps_p.tile([128, 128], F32, tag="pps", name="warmps")
            for _ in range(40):
                nc.tensor.matmul(warm_ps[:], lhsT=warm_w[:], rhs=warm_w[:],
                                 start=True, stop=True)

            # prime the ACT exp table during the initial DMA window
            warm = small_p.tile([1, 8], F32, tag="warm")
            nc.vector.memset(warm[:], 0.0)
            warm2 = small_p.tile([1, 8], F32, tag="warm2")
            nc.scalar.activation(warm2[:], warm[:], EXP, scale=1.0)

            # ---- input DMAs: one big 3D-AP transfer per tensor, ordered
            # for earliest first score block; small stuff on the scalar
            # (Activation) HWDGE queue so its issue overlaps.
            def load_w(w_d, name):
                w_sb = w_p.tile([128, KT * GD], BF16, tag="w", name=name)
                nc.sync.dma_start(w_sb[:], w_d.ap())
                return w_sb

            def x_views(pool, w, name, src_ap, blocks):
                """sbuf tile [128, KT*w] laid out (block, k, s-in-block);
                returns (slice accessor, per-block contiguous DMA loader)."""
                sb = pool.tile([128, KT * w], BF16, tag=name, name=name)
                offs = []
                acc = 0
                for (lo, hi) in blocks:
                    offs.append((lo, hi, acc))
                    acc += KT * (hi - lo)

                def ap(k, o, n):
                    for (lo, hi, base) in offs:
                        if lo <= o and o + n <= hi:
                            c = base + k * (hi - lo) + (o - lo)
                            return sb[:, c:c + n]
                    raise ValueError((name, o, n))

                def load(bi, eng):
                    lo, hi, base = offs[bi]
                    c0, c1 = base, base + KT * (hi - lo)
                    eng.dma_start(sb[:, c0:c1], src_ap[:, c0:c1])
                return ap, load

            # two HWDGE queues (sync + scalar) stream in parallel; the
            # scalar-queue issues happen long before any exp is enqueued.
            mpad_sb = small_p.tile([128, NJT], BF16, tag="mpad")
            nc.scalar.dma_start(mpad_sb[:], mpad_d.ap())
            jh = min(512, J)
            kblocks = [(0, jh)] + ([(jh, J)] if J > jh else [])
            xk_t, xk_load = x_views(xk_p, J, "xk", xk_d.ap(), kblocks)
            xq_t, xq_load = x_views(xq_p, S, "xq", xq_d.ap(),
                                    [(0, 512), (512, 1024), (1024, 2048)])
            xv_t, xv_load = x_views(xv_p, J, "xv", xv_d.ap(), [(0, J)])
            wk_sb = load_w(wk_d, "wk_sb")
            xk_load(0, nc.sync)

            def load_w2(w_d, name):
                w_sb = w_p.tile([128, KT * GD], BF16, tag="w", name=name)
                nc.scalar.dma_start(w_sb[:], w_d.ap())
                return w_sb

            wq_sb = load_w2(wq_d, "wq_sb")
            xq_load(0, nc.sync)
            xq_load(1, nc.scalar)
            xq_load(2, nc.sync)
            if J > jh:
                xk_load(1, nc.sync)
            wv_sb = load_w2(wv_d, "wv_sb")
            xv_load(0, nc.scalar)
            if use_bias:
                bq_c = small_p.tile([128, 2], F32, tag="biasq")
                nc.sync.dma_start(
                    bq_c[:].rearrange("p (o x) -> p o x", o=2),
                    bq_d.ap().rearrange("(o p) x -> p o x", p=128))
                bk_c = small_p.tile([128, 2], F32, tag="biask")
                nc.sync.dma_start(
                    bk_c[:].rearrange("p (o x) -> p o x", o=2),
                    bk_d.ap().rearrange("(o p) x -> p o x", p=128))
                bv_row = small_p.tile([1, GD], BF16, tag="bvrow")
                nc.sync.dma_start(bv_row[:], bv_d.ap())
                ones_row = small_p.tile([1, 128], BF16, tag="ones")
                nc.vector.memset(ones_row[:], 1.0)

            # ---- projections ----
            qT = {}   # per otile: [128, S] bf16  (o on partitions)
            kTt = {}  # per otile: [128, J] bf16

            def proj_pass(x_tiles, w_sb, dst, bias_col, ot, chunk_group,
                          on_scalar=False):
                """One k-outer accumulation pass over <=2 width-chunks."""
                ps = [pps_p.tile([128, 512], F32, tag="pps",
                                 name=f"pps{ot}{o0}") for (o0, _) in chunk_group]
                for k in range(KT):
                    lw = w_sb[:, k * GD + ot * 128:k * GD + (ot + 1) * 128]
                    for ci, (o0, n) in enumerate(chunk_group):
                        nc.tensor.matmul(ps[ci][:, 0:n], lhsT=lw,
                                         rhs=x_tiles(k, o0, n),
                                         start=(k == 0), stop=(k == KT - 1))
                for ci, (o0, n) in enumerate(chunk_group):
                    if use_bias:
                        nc.vector.tensor_scalar(
                            dst[:, o0:o0 + n], ps[ci][:, 0:n],
                            bias_col[:, ot:ot + 1], None,
                            mybir.AluOpType.add)
                    elif on_scalar:
                        nc.scalar.copy(dst[:, o0:o0 + n], ps[ci][:, 0:n])
                    else:
                        nc.vector.tensor_copy(dst[:, o0:o0 + n], ps[ci][:, 0:n])

            def proj_passes(x_tiles, w_sb, dst_map, bias_col, width, ot,
                            groups=None):
                dst = qk_p.tile([128, width], BF16,
                                tag="qt" if width == S else "kt",
                                name=f"qk{ot}")
                dst_map[ot] = dst
                if groups is None:
                    groups = _pairs(_chunks(width, 512))
                return [
                    (lambda cg=cg:
                     proj_pass(x_tiles, w_sb, dst, bias_col, ot, cg))
                    for cg in groups
                ]

            vext = [None] * NJT

            def v_wave(jts):
                """V projection (flipped) for a couple of j-tiles + V_ext
                assembly via one strided copy per tile."""
                ps = [pps_p.tile([128, GD], F32, tag="pps", name=f"ppsv{jt}")
                      for jt in jts]
                for k in range(KT):
                    for vi, jt in enumerate(jts):
                        nc.tensor.matmul(
                            ps[vi][:, :],
                            lhsT=xv_t(k, jt * 128, 128),
                            rhs=wv_sb[:, k * GD:(k + 1) * GD],
                            start=(k == 0),
                            stop=(k == KT - 1) and not use_bias)
                for vi, jt in enumerate(jts):
                    if use_bias:
                        nc.tensor.matmul(ps[vi][:, :], lhsT=ones_row[:, :],
                                         rhs=bv_row[:, :], start=False,
                                         stop=True)
                    ve = vext_p.tile([128, GH * VW1], BF16, tag="vext",
                                     name=f"vext{jt}")
                    ve3 = ve[:].rearrange("p (h w) -> p h w", h=GH)
                    nc.vector.tensor_copy(
                        ve3[:, :, 0:HD],
                        ps[vi][:].rearrange("p (h w) -> p h w", h=GH))
                    for h in range(GH):
                        nc.vector.tensor_copy(
                            ve[:, h * VW1 + HD:h * VW1 + HD + 1],
                            mpad_sb[:, jt:jt + 1])
                    vext[jt] = ve

            def emit_qkt(pidx, ib, hp, fillers, start_jt=1):
                """Scores + exp for head pair hp of i-block ib. exp runs on
                ACT or DVE (Schraudolph) per the policy table."""
                i0 = ib * IBLK
                pt = {}
                for jt in range(NJT):
                    psA = sps_p.tile([128, IBLK], F32, tag="sps",
                                     name=f"sA{ib}{hp}{jt}")
                    psB = sps_p.tile([128, IBLK], F32, tag="sps",
                                     name=f"sB{ib}{hp}{jt}")
                    for (o, n) in _chunks(IBLK, 512):
                        nc.tensor.matmul(
                            psA[:, o:o + n],
                            lhsT=kTt[hp][0:64, jt * 128:(jt + 1) * 128],
                            rhs=qT[hp][0:64, i0 + o:i0 + o + n],
                            start=True, stop=True)
                        nc.tensor.matmul(
                            psB[:, o:o + n],
                            lhsT=kTt[hp][64:128, jt * 128:(jt + 1) * 128],
                            rhs=qT[hp][64:128, i0 + o:i0 + o + n],
                            start=True, stop=True)
                    for half, psX in ((0, psA), (1, psB)):
                        ptX = pt_p.tile([128, IBLK], BF16, tag="pt",
                                        name=f"pt{half}{ib}{hp}{jt}")
                        if policy[(pidx, jt, half)]:
                            nc.vector.tensor_scalar(
                                ptX[:].bitcast(I16), psX[:],
                                SCHR_C * scale, SCHR_D, ALU.mult, ALU.add)
                        else:
                            nc.scalar.activation(ptX[:], psX[:], EXP,
                                                 scale=scale)
                        pt[(hp * 2 + half, jt)] = ptX
                    if jt >= start_jt and fillers:
                        fillers.pop(0)()
                while fillers:
                    fillers.pop(0)()
                return pt

            def drain(pv, name_, h, lo, hi):
                osb = ost_p.tile([VW1, 512], F32, tag="ost", name=name_)
                nc.vector.tensor_copy(osb[:], pv[:, :])
                drain_ctr[0] += 1
                nc.sync.dma_start(out_d.ap()[h * VW1:(h + 1) * VW1, lo:hi],
                                  osb[:])

            def pv_group(ib, hp, hl, pt):
                """PV accumulation for one head, both i-chunks in lockstep
                per j-tile (one vext LDW covers two matmuls); numerators +
                denominator row DMA'd out unnormalized (host divides)."""
                h = hp * 2 + hl
                pvs = [pps_p.tile([VW1, 512], F32, tag="pps",
                                  name=f"pv{ib}{h}{icl}")
                       for icl in range(IBLK // 512)]
                for jt in range(NJT):
                    for icl in range(IBLK // 512):
                        nc.tensor.matmul(
                            pvs[icl][:, :],
                            lhsT=vext[jt][:, h * VW1:(h + 1) * VW1],
                            rhs=pt[(h, jt)][:, icl * 512:(icl + 1) * 512],
                            start=(jt == 0), stop=(jt == NJT - 1))
                for icl in range(IBLK // 512):
                    drain(pvs[icl], f"o{ib}{h}{icl}", h,
                          ib * IBLK + icl * 512, ib * IBLK + (icl + 1) * 512)

            def pv_fillers(ib, hp, pt):
                return [(lambda hl=hl: pv_group(ib, hp, hl, pt))
                        for hl in range(2)]

            def pv_tail(ib, hp, pt):
                """Final-phase PV: the two icl-groups of each head accumulate
                in lockstep per j-tile so only the last j-tile's matmuls
                trail the final exps."""
                for hl in range(2):
                    h = hp * 2 + hl
                    pvs = [pps_p.tile([VW1, 512], F32, tag="pps",
                                      name=f"pvt{h}{icl}")
                           for icl in range(IBLK // 512)]
                    for jt in range(NJT):
                        for icl in range(IBLK // 512):
                            nc.tensor.matmul(
                                pvs[icl][:, :],
                                lhsT=vext[jt][:, h * VW1:(h + 1) * VW1],
                                rhs=pt[(h, jt)][:, icl * 512:(icl + 1) * 512],
                                start=(jt == 0), stop=(jt == NJT - 1))
                    for icl in range(IBLK // 512):
                        drain(pvs[icl], f"ot{h}{icl}", h,
                              ib * IBLK + icl * 512,
                              ib * IBLK + (icl + 1) * 512)

            # ---- emission schedule ----
            bqc = bq_c if use_bias else None
            bkc = bk_c if use_bias else None
            kc = _chunks(J, 512)
            k0_passes = proj_passes(xk_t, wk_sb, kTt, bkc, J, 0,
                                    groups=[[kc[0]], kc[1:]])
            k0_passes[0]()   # j 0:512 -> scores jt0..3 can start
            # keep HAM warm across the xq DMA wait
            warm2_ps = pps_p.tile([128, 128], F32, tag="pps", name="warm2ps")
            for _ in range(20):
                nc.tensor.matmul(warm2_ps[:], lhsT=warm_w[:], rhs=warm_w[:],
                                 start=True, stop=True)
            q0_passes = proj_passes(xq_t, wq_sb, qT, bqc, S, 0)
            q0_passes[0]()   # i 0:1024 -> first score block can start

            v_fillers = [(lambda js=js: v_wave(js))
                         for js in _pairs(list(range(NJT)))]
            q1_fillers = proj_passes(xq_t, wq_sb, qT, bqc, S, 1)
            k1_fillers = proj_passes(xk_t, wk_sb, kTt, bkc, J, 1)

            if pipelined:
                pt00 = emit_qkt(0, 0, 0,
                                k0_passes[1:] + q0_passes[1:] + q1_fillers,
                                start_jt=2)
                g00 = pv_fillers(0, 0, pt00)
                pt10 = emit_qkt(1, 1, 0, k1_fillers + v_fillers,
                                start_jt=2)
                g10 = pv_fillers(1, 0, pt10)
                pt01 = emit_qkt(2, 0, 1, g00 + g10[:1], start_jt=2)
                g01 = pv_fillers(0, 1, pt01)
                pt11 = emit_qkt(3, 1, 1, g10[1:] + g01, start_jt=2)
                for f in pv_fillers(1, 1, pt11):
                    f()
            else:
                for p in k0_passes[1:] + q0_passes[1:]:
                    p()
                pt00 = emit_qkt(0, 0, 0, [])
                for f in v_fillers + q1_fillers + k1_fillers:
                    f()
                for f in pv_fillers(0, 0, pt00):
                    f()
                pt10 = emit_qkt(1, 1, 0, [])
                for f in pv_fillers(1, 0, pt10):
                    f()
                pt01 = emit_qkt(2, 0, 1, [])
                for f in pv_fillers(0, 1, pt01):
                    f()
                pt11 = emit_qkt(3, 1, 1, [])
                for f in pv_fillers(1, 1, pt11):
                    f()

    nc.compile()
    return nc


def _w_prep(w):
    """(GD, D) torch-layout weight slice -> [128, KT*GD] sbuf layout."""
    bf = ml_dtypes.bfloat16
    wt = np.ascontiguousarray(w.T)            # (D, GD)
    return np.ascontiguousarray(
        wt.reshape(KT, 128, GD).transpose(1, 0, 2).reshape(128, KT * GD)
    ).astype(bf)


def _x_prep(x, blocks):
    """(seq, D) activation -> [128, (block, k, s)] tiled layout matching
    the kernel's x_views blocks."""
    bf = ml_dtypes.bfloat16
    xt = np.ascontiguousarray(x.T)            # (D, seq)
    t = xt.reshape(KT, 128, xt.shape[1])
    parts = [np.ascontiguousarray(t[:, :, lo:hi].transpose(1, 0, 2))
             .reshape(128, KT * (hi - lo)) for (lo, hi) in blocks]
    return np.ascontiguousarray(np.concatenate(parts, axis=1)).astype(bf)


def _prep_and_run(inputs, trace=False):
    query = np.asarray(inputs["query"], dtype=np.float32)
    key = np.asarray(inputs["key"], dtype=np.float32)
    value = np.asarray(inputs["value"], dtype=np.float32)
    mask = np.asarray(inputs["mask"]).reshape(S)
    w_q = np.asarray(inputs["w_q"], dtype=np.float32)
    b_q = np.asarray(inputs["b_q"], dtype=np.float32)
    w_k = np.asarray(inputs["w_k"], dtype=np.float32)
    b_k = np.asarray(inputs["b_k"], dtype=np.float32)
    w_v = np.asarray(inputs["w_v"], dtype=np.float32)
    b_v = np.asarray(inputs["b_v"], dtype=np.float32)

    use_bias = bool(np.any(b_q) or np.any(b_k) or np.any(b_v))

    # compact key/value over masked-out positions
    idx = np.nonzero(mask != 0)[0]
    J_real = int(len(idx))
    assert J_real > 0, "all positions masked: softmax undefined"
    J = max(512, ((J_real + 127) // 128) * 128)
    key_c = np.zeros((J, B, D), np.float32)
    key_c[:J_real] = key[idx]
    value_c = np.zeros((J, B, D), np.float32)
    value_c[:J_real] = value[idx]

    bf = ml_dtypes.bfloat16
    NJT = J // 128
    mflat = np.zeros(J, np.float32)
    mflat[:J_real] = 1  # mpad[p, t] = 1 iff t*128+p < J_real
    mpad = np.ascontiguousarray(mflat.reshape(NJT, 128).T).astype(bf)
    in_maps = []
    for core in range(N_CORES):
        b = core // GROUPS
        g = core % GROUPS
        hs = slice(g * GD, (g + 1) * GD)
        jh = min(512, J)
        kblocks = [(0, jh)] + ([(jh, J)] if J > jh else [])
        m = {
            "xq": _x_prep(query[:, b, :],
                          [(0, 512), (512, 1024), (1024, 2048)]),
            "xk": _x_prep(key_c[:, b, :], kblocks),
            "xv": _x_prep(value_c[:, b, :], [(0, J)]),
            "wq": _w_prep(w_q[hs, :]),
            "wk": _w_prep(w_k[hs, :]),
            "wv": _w_prep(w_v[hs, :]),
            "mpad": mpad,
        }
        if use_bias:
            m["bq"] = np.ascontiguousarray(b_q[hs]).reshape(GD, 1)
            m["bk"] = np.ascontiguousarray(b_k[hs]).reshape(GD, 1)
            m["bv"] = np.ascontiguousarray(b_v[hs]).reshape(1, GD).astype(bf)
        in_maps.append(m)

    ck = (J, J_real, use_bias)
    if ck not in _CACHE:
        _CACHE[ck] = _build(J, J_real, use_bias)
    nc = _CACHE[ck]

    kwargs = {}
    if trace:
        kwargs = dict(trace=True, trace_cores=list(range(N_CORES)))
    res = run_bass_kernel_spmd(nc, in_maps, core_ids=list(range(N_CORES)),
                               **kwargs)

    out = np.empty((S, B, D), np.float32)
    for core in range(N_CORES):
        b = core // GROUPS
        g = core % GROUPS
        r = res.results[core]["out"].reshape(GH, VW1, S)
        out[:, b, g * GD:(g + 1) * GD] = (
            (r[:, :HD, :] / r[:, HD:HD + 1, :])     # softmax denominator
            .reshape(GD, S).T)
    return out, res


def kernel(**inputs):
    out, _ = _prep_and_run(inputs, trace=False)
    return out


def run_traced(**inputs):
    _, res = _prep_and_run(inputs, trace=True)
    return res
